# revision 6
# baseline (speedup 1.0000x reference)
"""Trainium2 Bass kernel for KnowledgeAwareCLIPLoss.

For each pair (e1, e2) in train_ill:
    align  = -log_sigmoid(cos(img[e1], txt[e2]) + cos(img[e1], img[e2]) + cos(txt[e1], txt[e2]))
    name   = -log_sigmoid(cos(nam[e1], nam[e2]))
    graph  = -log_sigmoid(cos(grf[e1], grf[e2]))
loss = (sum(align) + 0.1*sum(name) + 0.1*sum(graph)) / (3*M)

Strategy (memory-bound gather problem):
  - Host L2-normalizes every table row (eps clamp folded in) so cosines
    become plain dots, and quantizes to fp8 (the loss averages 300k terms,
    so fp8 noise washes out).
  - Side 1 rows come from an augmented table aug1[i] = [img|txt|img|txt|nam|grf]
    (3072 fp8) fetched with dma_gather — pairs are pre-sorted into 4 buckets
    by e1-shard so in-shard indices fit int16; one gpsimd call covers ~1000
    rows, keeping the Q7 descriptor generator off the critical path.
  - Side 2 rows are added IN-FLIGHT by the DMA engine: one indirect gather
    per group with compute_op=add (CCE) accumulates the plain 2048-wide row
    [b_i|b_t|b_n|b_g] onto X[1024:3072], producing
    X = [a_i | a_t | a_i+b_i | a_t+b_t | a_n+b_n | a_g+b_g].
    (CCE descriptors are capped at 2048 elements, which this respects.)
  - Dots from sums-of-squares: |u+v|^2 = 2 + 2*cos for normalized rows, so
    ACT Square+accumulate covers d2+d3 and d4 while DVE handles d5 and the
    misaligned d1 = a_i.(a_t+b_t) - a_i.a_t as two fused dots.
  - -log_sigmoid(x) = ln(1 + exp(-x)): Exp and Ln share one ACT table.
  - Device writes [128, n_groups, 3] softplus partials; host does the
    masked weighted sum across cores (the scalar all-reduce) and division.
"""

import sys

if "/opt/trn_rl_repo" not in sys.path:
    sys.path.insert(0, "/opt/trn_rl_repo")

import numpy as np

N = 100000          # entities
D = 512             # embedding dim
M = 100000          # pairs
N_CORES = 8
P = 128             # pairs per group (SBUF partitions)
DT = 4 * D          # plain interleaved row width (2048)
WA = 6 * D          # augmented side-1 row width (3072)
NSHARD = 4          # side-1 shards (int16 index range for dma_gather)
SH = N // NSHARD    # 25000 rows per shard
CHUNK_MAX = 1024    # max idxs per dma_gather call (8 groups)
KNOWLEDGE_WEIGHT = 0.1
EPS = 1e-8

TRACE = False        # set True (e.g. from test.py) to NTFF-profile the run
LAST_EXEC_NS = None  # exec time of the last traced run

_CACHE = {}

NC_COLS = 6  # per-group columns: [t1, t2, Scc, xa, S4, S5]


def _emit(tc, nc, aug1, table2, idx1, idx2, out_dram, chunks, n_groups):
    """Per-core program. chunks: list of (r1, g0, k)."""
    from contextlib import ExitStack

    import concourse.bass as bass
    from concourse import library_config, mybir

    f32 = mybir.dt.float32
    bf16 = mybir.dt.bfloat16
    fp8 = mybir.dt.float8e4
    AF = mybir.ActivationFunctionType
    Alu = mybir.AluOpType
    kmax = max(k for _, _, k in chunks)
    n_idx16 = (n_groups * P) // 16

    with ExitStack() as ctx:
        singles = ctx.enter_context(tc.tile_pool(name="singles", bufs=1))
        gather_pool = ctx.enter_context(tc.tile_pool(name="gather", bufs=4))
        scratch = ctx.enter_context(tc.tile_pool(name="scratch", bufs=2))
        small = ctx.enter_context(tc.tile_pool(name="small", bufs=2))

        nc.gpsimd.load_library(library_config.mlp)

        idx1_sb = singles.tile([P, n_idx16], mybir.dt.int16)
        idx2_sb = singles.tile([P, n_groups], mybir.dt.int32)
        nc.sync.dma_start(out=idx1_sb[:], in_=idx1[:])
        nc.sync.dma_start(out=idx2_sb[:], in_=idx2[:])

        bias2 = singles.tile([P, 1], f32)
        nc.gpsimd.memset(bias2[:], 2.0)

        # flat so accum_out slices are 2-D; viewed 3-D for reduce/ACT
        Dt = singles.tile([P, n_groups * NC_COLS], f32)
        Dtv = Dt.rearrange("p (g c) -> p g c", c=NC_COLS)
        sp = singles.tile([P, n_groups, 3], f32)  # softplus outputs

        def col(g, c):
            return Dt[:, g * NC_COLS + c : g * NC_COLS + c + 1]

        for r1, g0, k in chunks:
            nk = k * P
            o16 = (g0 * P) // 16
            X = gather_pool.tile([P, kmax, WA], fp8, tag="X")
            nc.gpsimd.dma_gather(
                X[:, 0:k, :], aug1[r1 * SH : (r1 + 1) * SH, :],
                idx1_sb[:, o16 : o16 + nk // 16], nk, nk, WA)
            for j in range(k):
                g = g0 + j
                # CCE: X[:, j, 1024:3072] += table2[e2[g]]
                nc.gpsimd.indirect_dma_start(
                    out=X[:, j, 2 * D : 6 * D], out_offset=None,
                    in_=table2[:],
                    in_offset=bass.IndirectOffsetOnAxis(
                        ap=idx2_sb[:, g : g + 1], axis=0),
                    compute_op=Alu.add)

            for j in range(k):
                g = g0 + j
                # t1 = a_i . (a_t + b_t), t2 = a_i . a_t  (d1 = t1 - t2)
                prod = scratch.tile([P, D], bf16, tag="tt")
                nc.vector.scalar_tensor_tensor(
                    out=prod[:], in0=X[:, j, 0:D], scalar=1.0,
                    in1=X[:, j, 3 * D : 4 * D], op0=Alu.mult, op1=Alu.mult,
                    accum_out=col(g, 0))
                prod = scratch.tile([P, D], bf16, tag="tt")
                nc.vector.scalar_tensor_tensor(
                    out=prod[:], in0=X[:, j, 0:D], scalar=1.0,
                    in1=X[:, j, D : 2 * D], op0=Alu.mult, op1=Alu.mult,
                    accum_out=col(g, 1))
                # S5 = |a_g + b_g|^2 = 2 + 2*d5 (DVE)
                prod = scratch.tile([P, D], bf16, tag="tt")
                nc.vector.scalar_tensor_tensor(
                    out=prod[:], in0=X[:, j, 5 * D : 6 * D], scalar=1.0,
                    in1=X[:, j, 5 * D : 6 * D], op0=Alu.mult, op1=Alu.mult,
                    accum_out=col(g, 5))
                # Scc = |a_i+b_i|^2 + |a_t+b_t|^2 = 4 + 2*(d2+d3)  (ACT)
                sq = scratch.tile([P, 2 * D], bf16, tag="sq")
                nc.scalar.activation(
                    out=sq[:], in_=X[:, j, 2 * D : 4 * D], func=AF.Square,
                    accum_out=col(g, 2))
                # S4 = |a_n + b_n|^2 = 2 + 2*d4 (ACT)
                sq = scratch.tile([P, 2 * D], bf16, tag="sq")
                nc.scalar.activation(
                    out=sq[:, 0:D], in_=X[:, j, 4 * D : 5 * D], func=AF.Square,
                    accum_out=col(g, 4))

            # xa = (t1 - t2) + 0.5*Scc - 2   [the -2 folds into Exp's bias]
            tmp = small.tile([P, kmax, 1], f32, tag="tmp")
            nc.vector.tensor_tensor(
                out=tmp[:, 0:k, :], in0=Dtv[:, g0 : g0 + k, 0:1],
                in1=Dtv[:, g0 : g0 + k, 1:2], op=Alu.subtract)
            nc.vector.scalar_tensor_tensor(
                out=Dtv[:, g0 : g0 + k, 3:4], in0=Dtv[:, g0 : g0 + k, 2:3],
                scalar=0.5, in1=tmp[:, 0:k, :], op0=Alu.mult, op1=Alu.add)
            # softplus(-x) = ln(1 + exp(-x)); biases fold the -2/-1 shifts:
            # exp(-(xa'-2)) = exp(-xa'+2); exp(-(S-2)/2) = exp(-S/2+1)
            E = small.tile([P, kmax, 3], f32, tag="E")
            nc.scalar.activation(
                out=E[:, 0:k, 0:1], in_=Dtv[:, g0 : g0 + k, 3:4], func=AF.Exp,
                scale=-1.0, bias=bias2[:])
            nc.scalar.activation(
                out=E[:, 0:k, 1:3], in_=Dtv[:, g0 : g0 + k, 4:6], func=AF.Exp,
                scale=-0.5, bias=1.0)
            nc.scalar.activation(
                out=sp[:, g0 : g0 + k, :], in_=E[:, 0:k, :], func=AF.Ln,
                bias=1.0)

        nc.sync.dma_start(out=out_dram[:], in_=sp[:])


def _build(chunks, n_groups, n_cores=N_CORES):
    """Build + compile the SPMD program for a given chunk structure."""
    from concourse import bacc, mybir, tile

    nc = bacc.Bacc(
        "TRN2",
        target_bir_lowering=False,
        debug=False,
        enable_asserts=False,
        num_devices=n_cores,
    )
    f32 = mybir.dt.float32
    fp8 = mybir.dt.float8e4
    n_idx16 = (n_groups * P) // 16
    aug1 = nc.dram_tensor("aug1", [N, WA], fp8, kind="ExternalInput").ap()
    table2 = nc.dram_tensor("table2", [N, DT], fp8, kind="ExternalInput").ap()
    idx1 = nc.dram_tensor(
        "idx1", [P, n_idx16], mybir.dt.int16, kind="ExternalInput").ap()
    idx2 = nc.dram_tensor(
        "idx2", [P, n_groups], mybir.dt.int32, kind="ExternalInput").ap()
    out = nc.dram_tensor(
        "out", [P, n_groups, 3], f32, kind="ExternalOutput").ap()

    with tile.TileContext(nc) as tc:
        _emit(tc, nc, aug1, table2, idx1, idx2, out, chunks, n_groups)
    nc.compile()
    return nc


def _wrap_idx(vals):
    """dma_gather index layout: idx i -> [i % 16, i // 16], replicated to
    128 partitions (8 Q7 cores x 16)."""
    w = vals.reshape(-1, 16).T  # [16, n/16]
    return np.tile(w, (8, 1)).astype(np.int16)


def kernel(img_emb, text_emb, entity_names, graph_emb, train_ill):
    global LAST_EXEC_NS
    from concourse.bass_utils import run_bass_kernel_spmd

    import ml_dtypes

    train_ill = np.asarray(train_ill)

    # L2-normalized fp8 tables.
    norm = []
    for t in (img_emb, text_emb, entity_names, graph_emb):
        t = np.asarray(t, dtype=np.float32)
        norms = np.sqrt(np.einsum("nd,nd->n", t, t, dtype=np.float32))
        norm.append((t / np.maximum(norms, EPS)[:, None]).astype(
            ml_dtypes.float8_e4m3))
    img_n, txt_n, nam_n, grf_n = norm
    aug1 = np.concatenate([img_n, txt_n, img_n, txt_n, nam_n, grf_n], axis=1)
    table2 = np.concatenate([img_n, txt_n, nam_n, grf_n], axis=1)

    e1 = train_ill[:, 0].astype(np.int64)
    e2 = train_ill[:, 1].astype(np.int64)

    # Sort pairs into 4 buckets by e1-shard; pad each bucket to a multiple
    # of 8*128 so all cores get identical chunk shapes.
    b = e1 // SH
    order = np.argsort(b, kind="stable")
    e1s, e2s, bs = e1[order], e2[order], b[order]
    counts = np.bincount(bs, minlength=NSHARD)
    padded = ((counts + N_CORES * P - 1) // (N_CORES * P)) * (N_CORES * P)
    K = padded // N_CORES                       # per-core slots per bucket
    S = int(K.sum())                            # per-core total slots
    n_groups = S // P

    idx1_pc = np.zeros((N_CORES, S), np.int64)
    idx2_pc = np.zeros((N_CORES, S), np.int64)
    valid_pc = np.zeros((N_CORES, S), bool)
    chunks = []
    pos = 0
    g0 = 0
    slot0 = 0
    for bk in range(NSHARD):
        nb, kb = int(counts[bk]), int(K[bk])
        if kb == 0:
            continue
        l1 = np.zeros(kb * N_CORES, np.int64)
        l2 = np.zeros(kb * N_CORES, np.int64)
        l1[:nb] = e1s[pos : pos + nb] - bk * SH
        l2[:nb] = e2s[pos : pos + nb]
        for c in range(N_CORES):
            idx1_pc[c, slot0 : slot0 + kb] = l1[c * kb : (c + 1) * kb]
            idx2_pc[c, slot0 : slot0 + kb] = l2[c * kb : (c + 1) * kb]
            nv = min(max(nb - c * kb, 0), kb)
            valid_pc[c, slot0 : slot0 + nv] = True
        left, gg = kb, g0
        while left > 0:
            take = min(left, CHUNK_MAX)
            chunks.append((bk, gg, take // P))
            gg += take // P
            left -= take
        g0 += kb // P
        slot0 += kb
        pos += nb
    assert slot0 == S and pos == M

    key = (n_groups, tuple(chunks))
    if _CACHE.get("key") != key:
        _CACHE["nc"] = _build(chunks, n_groups)
        _CACHE["key"] = key
    nc = _CACHE["nc"]

    in_maps = [
        {
            "aug1": aug1,
            "table2": table2,
            "idx1": _wrap_idx(idx1_pc[c]),
            "idx2": np.ascontiguousarray(
                idx2_pc[c].reshape(n_groups, P).T.astype(np.int32)),
        }
        for c in range(N_CORES)
    ]
    res = run_bass_kernel_spmd(nc, in_maps, list(range(N_CORES)), trace=TRACE)
    if TRACE:
        LAST_EXEC_NS = res.exec_time_ns

    # Host unshard: masked weighted sum of softplus(-x) = -ln(sigmoid(x)).
    total = 0.0
    for c in range(N_CORES):
        o = np.asarray(res.results[c]["out"], dtype=np.float64)  # [P, G, 3]
        mask = valid_pc[c].reshape(n_groups, P).T[:, :, None]    # [P, G, 1]
        o = o * mask
        total += o[:, :, 0].sum() + KNOWLEDGE_WEIGHT * (
            o[:, :, 1].sum() + o[:, :, 2].sum()
        )
    loss = total / (3 * M)
    return np.float32(loss)


# revision 7
# speedup vs baseline: 1.1695x; 1.1695x over previous
"""Trainium2 Bass kernel for KnowledgeAwareCLIPLoss.

For each pair (e1, e2) in train_ill:
    align  = -log_sigmoid(cos(img[e1], txt[e2]) + cos(img[e1], img[e2]) + cos(txt[e1], txt[e2]))
    name   = -log_sigmoid(cos(nam[e1], nam[e2]))
    graph  = -log_sigmoid(cos(grf[e1], grf[e2]))
loss = (sum(align) + 0.1*sum(name) + 0.1*sum(graph)) / (3*M)

Strategy (memory-bound gather problem):
  - Host L2-normalizes every row of the 4 embedding tables (with the
    reference's eps clamp folded in) and interleaves them into one
    [N, 4*D] fp8 array. Cosines then reduce to plain dots, and fp8 halves
    the gather traffic (the final loss averages 300k terms, so fp8
    quantization noise washes out).
  - Pairs are data-parallel sharded across 8 cores (12500 each), processed
    in groups of 128 (one SBUF partition per pair), 7 groups per gather
    batch, double-buffered 4 deep so SWDGE descriptor generation, SDMA
    transfers and DVE dots overlap.
  - Dots via fused DVE scalar_tensor_tensor (single pass over the fp8
    operands with an f32 accumulator — no materialized product + second
    reduce pass). img/txt blocks are adjacent, so
    cos(img1,img2)+cos(txt1,txt2) is a single 1024-wide dot.
  - -log_sigmoid(x) = softplus(-x) = ln(1 + exp(-x)): Exp and Ln live in
    the same ACT function table, so table reloads stay off the DVE path.
  - Device writes [128, n_groups, 3] softplus partials; host does the
    masked weighted sum across cores (the scalar all-reduce) and division.
"""

import sys

if "/opt/trn_rl_repo" not in sys.path:
    sys.path.insert(0, "/opt/trn_rl_repo")

import numpy as np

N = 100000          # entities
D = 512             # embedding dim
M = 100000          # pairs
N_CORES = 8
P = 128             # pairs per group (SBUF partitions)
PAIRS_PER_CORE = M // N_CORES            # 12500
N_GROUPS = (PAIRS_PER_CORE + P - 1) // P  # 98
DT = 4 * D          # interleaved row width (2048)
KD = 7              # groups per gather batch (98 = 14 * 7)
NB = N_GROUPS // KD  # 14 gather batches
KNOWLEDGE_WEIGHT = 0.1
EPS = 1e-8

TRACE = False        # set True (e.g. from test.py) to NTFF-profile the run
LAST_EXEC_NS = None  # exec time of the last traced run

_CACHE = {}

NC_COLS = 5  # per-group dot columns: [d1, d23, xa, d4, d5]


def _emit(tc, nc, table, idx1, idx2, out_dram, n_groups):
    """Per-core program: 14 batches of 7x128 pairs; fused fp8 dots."""
    from contextlib import ExitStack

    import concourse.bass as bass
    from concourse import mybir

    f32 = mybir.dt.float32
    bf16 = mybir.dt.bfloat16
    fp8 = mybir.dt.float8e4
    AF = mybir.ActivationFunctionType
    Alu = mybir.AluOpType
    X = mybir.AxisListType.X

    with ExitStack() as ctx:
        singles = ctx.enter_context(tc.tile_pool(name="singles", bufs=1))
        gather_pool = ctx.enter_context(tc.tile_pool(name="gather", bufs=4))
        scratch = ctx.enter_context(tc.tile_pool(name="scratch", bufs=2))
        small = ctx.enter_context(tc.tile_pool(name="small", bufs=2))

        idx1_sb = singles.tile([P, n_groups], mybir.dt.int32)
        idx2_sb = singles.tile([P, n_groups], mybir.dt.int32)
        nc.sync.dma_start(out=idx1_sb[:], in_=idx1[:])
        nc.sync.dma_start(out=idx2_sb[:], in_=idx2[:])

        # flat so accum_out slices are 2-D; viewed 3-D for reduce/ACT
        Dt = singles.tile([P, n_groups * NC_COLS], f32)
        Dtv = Dt.rearrange("p (g c) -> p g c", c=NC_COLS)
        sp = singles.tile([P, n_groups, 3], f32)  # softplus outputs

        # (col, a_off, b_off, width)
        dots = [
            (0, 0, D, D),          # d1  = img1 . txt2
            (1, 0, 0, 2 * D),      # d23 = img1.img2 + txt1.txt2
            (3, 2 * D, 2 * D, D),  # d4  = nam1 . nam2
            (4, 3 * D, 3 * D, D),  # d5  = grf1 . grf2
        ]

        for nb in range(NB):
            g0 = nb * KD
            A = gather_pool.tile([P, KD, DT], fp8, tag="A")
            B = gather_pool.tile([P, KD, DT], fp8, tag="B")
            for j in range(KD):
                # one 128-row gather per call: the runtime's SWDGE path
                # only supports a single index per partition
                nc.gpsimd.indirect_dma_start(
                    out=A[:, j, :], out_offset=None, in_=table[:],
                    in_offset=bass.IndirectOffsetOnAxis(
                        ap=idx1_sb[:, g0 + j : g0 + j + 1], axis=0),
                )
                nc.gpsimd.indirect_dma_start(
                    out=B[:, j, :], out_offset=None, in_=table[:],
                    in_offset=bass.IndirectOffsetOnAxis(
                        ap=idx2_sb[:, g0 + j : g0 + j + 1], axis=0),
                )

            for j in range(KD):
                g = g0 + j
                for c, ao, bo, w in dots:
                    prod = scratch.tile([P, 2 * D], bf16, tag="tt")
                    # fused dot: out=(in0*1)*in1, accum_out=sum(out)
                    nc.vector.scalar_tensor_tensor(
                        out=prod[:, 0:w],
                        in0=A[:, j, ao : ao + w],
                        scalar=1.0,
                        in1=B[:, j, bo : bo + w],
                        op0=Alu.mult,
                        op1=Alu.mult,
                        accum_out=Dt[:, g * NC_COLS + c : g * NC_COLS + c + 1],
                    )

            # xa = d1 + d23, then softplus(-x) = ln(1 + exp(-x))
            nc.vector.tensor_reduce(
                out=Dtv[:, g0 : g0 + KD, 2:3],
                in_=Dtv[:, g0 : g0 + KD, 0:2], axis=X, op=Alu.add)
            E = small.tile([P, KD, 3], f32, tag="E")
            nc.scalar.activation(
                out=E[:], in_=Dtv[:, g0 : g0 + KD, 2:5], func=AF.Exp,
                scale=-1.0)
            nc.scalar.activation(
                out=sp[:, g0 : g0 + KD, :], in_=E[:], func=AF.Ln, bias=1.0)

        nc.sync.dma_start(out=out_dram[:], in_=sp[:])


def _build(n_rows, n_groups, n_cores=N_CORES):
    """Build + compile the SPMD program. Returns the Bacc module."""
    from concourse import bacc, mybir, tile

    nc = bacc.Bacc(
        "TRN2",
        target_bir_lowering=False,
        debug=False,
        enable_asserts=False,
        num_devices=n_cores,
    )
    f32 = mybir.dt.float32
    table = nc.dram_tensor(
        "table", [n_rows, DT], mybir.dt.float8e4, kind="ExternalInput").ap()
    idx1 = nc.dram_tensor(
        "idx1", [P, n_groups], mybir.dt.int32, kind="ExternalInput").ap()
    idx2 = nc.dram_tensor(
        "idx2", [P, n_groups], mybir.dt.int32, kind="ExternalInput").ap()
    out = nc.dram_tensor(
        "out", [P, n_groups, 3], f32, kind="ExternalOutput").ap()

    with tile.TileContext(nc) as tc:
        _emit(tc, nc, table, idx1, idx2, out, n_groups)
    nc.compile()
    return nc


def _get_full_nc():
    if "nc" not in _CACHE:
        _CACHE["nc"] = _build(N, N_GROUPS)
    return _CACHE["nc"]


def _make_inputs_per_core(table, e1, e2, core):
    """Index layout for one core: pair k of the core -> slot (p=k%128, g=k//128)."""
    k0 = core * PAIRS_PER_CORE
    pad = N_GROUPS * P
    i1 = np.zeros(pad, np.int32)
    i2 = np.zeros(pad, np.int32)
    i1[:PAIRS_PER_CORE] = e1[k0 : k0 + PAIRS_PER_CORE]
    i2[:PAIRS_PER_CORE] = e2[k0 : k0 + PAIRS_PER_CORE]
    return {
        "table": table,
        "idx1": np.ascontiguousarray(i1.reshape(N_GROUPS, P).T),
        "idx2": np.ascontiguousarray(i2.reshape(N_GROUPS, P).T),
    }


def kernel(img_emb, text_emb, entity_names, graph_emb, train_ill):
    global LAST_EXEC_NS
    from concourse.bass_utils import run_bass_kernel_spmd

    import ml_dtypes

    train_ill = np.asarray(train_ill)

    # Interleaved L2-normalized fp8 table: row i = [img | txt | names | graph].
    # cos(a, b) == dot(a / max(|a|, eps), b / max(|b|, eps)) exactly.
    table = np.empty((N, DT), ml_dtypes.float8_e4m3)
    for t_i, t in enumerate((img_emb, text_emb, entity_names, graph_emb)):
        t = np.asarray(t, dtype=np.float32)
        norms = np.sqrt(np.einsum("nd,nd->n", t, t, dtype=np.float32))
        tn = t / np.maximum(norms, EPS)[:, None]
        table[:, t_i * D : (t_i + 1) * D] = tn.astype(ml_dtypes.float8_e4m3)

    e1 = train_ill[:, 0].astype(np.int32)
    e2 = train_ill[:, 1].astype(np.int32)

    in_maps = [_make_inputs_per_core(table, e1, e2, c) for c in range(N_CORES)]

    nc = _get_full_nc()
    res = run_bass_kernel_spmd(nc, in_maps, list(range(N_CORES)), trace=TRACE)
    if TRACE:
        LAST_EXEC_NS = res.exec_time_ns

    # Host unshard: masked weighted sum of softplus(-x) = -ln(sigmoid(x)).
    slot_pair = np.arange(N_GROUPS)[None, :] * P + np.arange(P)[:, None]
    valid = (slot_pair < PAIRS_PER_CORE).astype(np.float64)[:, :, None]
    total = 0.0
    for c in range(N_CORES):
        o = np.asarray(res.results[c]["out"], dtype=np.float64) * valid
        total += o[:, :, 0].sum() + KNOWLEDGE_WEIGHT * (
            o[:, :, 1].sum() + o[:, :, 2].sum()
        )
    loss = total / (3 * M)
    return np.float32(loss)
